# revision 20
# baseline (speedup 1.0000x reference)
"""Expert-parallel MoE (Mixtral-style top-2 of 8 experts, SwiGLU) on 8 TRN2 cores.

Strategy: mixed-granularity expert sharding. Experts sorted by token load:
the 4 "big" experts are split into QUARTERS of the intermediate dim (each
quarter-shard on a different core-slot), the 4 "small" experts into HALVES.
Each core runs 3 uniform slots (SPMD single program):
  A1: quarter-I shard of big expert 0/1,  capacity CA1 = max(load b0, b1)
  A2: quarter-I shard of big expert 2/3,  capacity CA2 = max(load b2, b3)
  B:  half-I shard of a small expert,     capacity CB  = max(small loads)
Per-core weight bytes stay at 1/8 of the total; the host sums the 4 (big)
or 2 (small) down-proj partials per expert. Finer shards hug the actual
loads tighter than one-expert-per-core (304+268+252 quarter-col-units vs
2*304), cutting the PE floor.

Device kernel per core (all matmuls bf16, fp32 PSUM accumulation), 16 GU
steps: step s computes gate/up+SwiGLU for (A1 j=s | A2 j=s-8) and B j=s;
one PSUM accumulator per A-slot and two for B fold the first down-proj
chunks in as act[j-1] completes. Remaining chunks run post-GU, slot-major,
j ascending to match the w2b DMA arrival order.

DMA discipline: sync-engine dma_start costs ~650 ns of issue time, so the
stream needs few instructions early (ko-block split of the first gate tile,
xt on the scalar engine's queue in parallel) and is emitted in consumption
order. Output writebacks ride the gpsimd queue (B-phase: sync) so they never
queue behind the bulk weight stream.
"""

import os

import ml_dtypes
import numpy as np

import concourse.bass as bass
from concourse import bacc
import concourse.mybir as mybir
import concourse.tile as tile
from concourse.bass_utils import run_bass_kernel_spmd

P = 128
H = 2048          # hidden dim
I = 4096          # intermediate dim
IQ = I // 4       # big-expert shard: quarter intermediate
IH = I // 2       # small-expert shard: half intermediate
E = 8
N_CORES = 8
BF16 = mybir.dt.bfloat16
F32 = mybir.dt.float32

KO = H // P       # 16 contraction steps over hidden dim
NJQ = IQ // P     # 8 j-tiles per quarter shard
NJH = IH // P     # 16 j-tiles per half shard
NSTEP = 16        # GU steps (A1 j=0..7 at steps 0-7, A2 j=0..7 at 8-15, B j=s)
NH = H // P       # 16 output row chunks
XB = 4            # xt ko-blocks for the first slot

# set by kernel() for test harness introspection
last_results = None


def _build_nc(CA1: int, CA2: int, CB: int) -> bass.Bass:
    act_fn = mybir.ActivationFunctionType
    CT = CA1 + CA2 + CB
    WA = H - P          # w2b width for A slots (NC1=1 -> first 128 cols in w2a)
    WBW = H - 2 * P     # w2b width for B slot (NC1=2)

    nc = bacc.Bacc()
    xt1_d = nc.declare_dram_parameter("xt1", [P, KO, CA1], BF16, isOutput=False)
    xt2_d = nc.declare_dram_parameter("xt2", [P, KO, CA2], BF16, isOutput=False)
    xtb_d = nc.declare_dram_parameter("xtb", [P, KO, CB], BF16, isOutput=False)
    # 64 w13 tiles packed by host in consumption order:
    # step s: [A-gate, A-up, B-gate, B-up] where A = A1 (s<8) else A2
    w13_d = nc.declare_dram_parameter("w13", [NSTEP * 4, P, KO, P], BF16, isOutput=False)
    # per GU step: A-slot 128-col strip + B 256-col strip, packed
    w2a_d = nc.declare_dram_parameter("w2a", [NSTEP, P, 3 * P], BF16, isOutput=False)
    w2b1_d = nc.declare_dram_parameter("w2b1", [NJQ // 2, P, 2, WA], BF16, isOutput=False)
    w2b2_d = nc.declare_dram_parameter("w2b2", [NJQ // 2, P, 2, WA], BF16, isOutput=False)
    w2bb_d = nc.declare_dram_parameter("w2bb", [NJH // 2, P, 2, WBW], BF16, isOutput=False)
    y_d = nc.declare_dram_parameter("y", [NH, P, CT], BF16, isOutput=True)

    with tile.TileContext(nc) as tc:
        with (
            tc.tile_pool(name="xp", bufs=1) as xp,
            tc.tile_pool(name="w13p", bufs=8) as w13p,
            tc.tile_pool(name="w2ap", bufs=4) as w2ap,
            tc.tile_pool(name="w2bp", bufs=1) as w2bp,
            tc.tile_pool(name="actp", bufs=1) as actp,
            tc.tile_pool(name="silup", bufs=2) as silup,
            tc.tile_pool(name="outp", bufs=4) as outp,
            tc.tile_pool(name="psgu", bufs=2, space="PSUM") as psgu,
            tc.tile_pool(name="psacc", bufs=1, space="PSUM") as psacc,
        ):
            w13_tiles = {}  # flat tile index -> sbuf tile

            def dma_w13(ti, blocked=False):
                sb = w13p.tile([P, KO, P], BF16, tag="w13", name=f"w13_{ti}")
                src = w13_d[ti]
                if blocked:
                    kb = KO // XB
                    for b in range(XB):
                        nc.sync.dma_start(sb[:, b * kb:(b + 1) * kb, :], src[:, b * kb:(b + 1) * kb, :])
                        xt_sb = xp.tile([P, kb, CA1], BF16, tag=f"xt1_{b}", name=f"xt1_{b}")
                        nc.scalar.dma_start(xt_sb[:], xt1_d[:, b * kb:(b + 1) * kb, :])
                        xt1_tiles.append(xt_sb)
                else:
                    nc.sync.dma_start(sb[:], src)
                w13_tiles[ti] = sb

            def dma_xt(tiles, src_d, cap, tagp, b, nblk):
                kb = KO // nblk
                xt_sb = xp.tile([P, kb, cap], BF16, tag=f"{tagp}_{b}", name=f"{tagp}_{b}")
                nc.scalar.dma_start(xt_sb[:], src_d[:, b * kb:(b + 1) * kb, :])
                tiles.append(xt_sb)

            w2a_tiles = {}

            def dma_w2a(s):
                sb = w2ap.tile([P, 3 * P], BF16, tag="w2a", name=f"w2a_{s}")
                nc.sync.dma_start(sb[:], w2a_d[s])
                w2a_tiles[s] = sb

            w2b_tiles = {}

            def dma_w2b(slot, jp):
                src = (w2b1_d, w2b2_d, w2bb_d)[slot]
                wd = WA if slot < 2 else WBW
                sb = w2bp.tile([P, 2, wd], BF16, tag=f"w2b_{slot}_{jp}", name=f"w2b_{slot}_{jp}")
                nc.sync.dma_start(sb[:], src[jp])
                w2b_tiles[(slot, jp)] = sb

            # slot tables: (slot, col offset, capacity, n_chunks_in_acc)
            SLOTS = [(0, 0, CA1, 1), (1, CA1, CA2, 1), (2, CA1 + CA2, CB, 2)]

            def xt_slice(slot, ko):
                if slot == 0:
                    kb = KO // XB
                    return xt1_tiles[ko // kb][:, ko % kb, :]
                tiles = xt2_tiles if slot == 1 else xtb_tiles
                kb = KO // 2
                return tiles[ko // kb][:, ko % kb, :]

            # ---- priming: consumption-ordered, minimal instruction count
            xt1_tiles, xt2_tiles, xtb_tiles = [], [], []
            dma_w13(0, blocked=True)       # A1-gate j0 + xt1 blocks (scalar q)
            dma_w13(1)                     # A1-up j0
            dma_xt(xtb_tiles, xtb_d, CB, "xtb", 0, 2)
            dma_w13(2)                     # B-gate j0
            dma_xt(xtb_tiles, xtb_d, CB, "xtb", 1, 2)
            dma_w13(3)                     # B-up j0
            dma_w2a(0)
            dma_w2a(1)
            for ti in (4, 5, 6, 7):        # step-1 group
                dma_w13(ti)

            # persistent PSUM accumulators for the first chunks of each slot
            acc = {}
            for slot, _, cw, nacc in SLOTS:
                for c in range(nacc):
                    acc[(slot, c)] = psacc.tile(
                        [P, cw], F32, tag=f"acc{slot}{c}", name=f"acc_{slot}_{c}"
                    )

            act_tiles = {}

            def gu_slot(slot, cw, j, ti_g, ti_u):
                g_ps = psgu.tile([P, cw], F32, tag="g", name=f"g_{slot}_{j}")
                u_ps = psgu.tile([P, cw], F32, tag="u", name=f"u_{slot}_{j}")
                for ti, ps in ((ti_g, g_ps), (ti_u, u_ps)):
                    w_sb = w13_tiles[ti]
                    for ko in range(KO):
                        nc.tensor.matmul(
                            ps[:], w_sb[:, ko, :], xt_slice(slot, ko),
                            start=(ko == 0), stop=(ko == KO - 1),
                        )
                s_sb = silup.tile([P, cw], F32, tag="s", name=f"s_{slot}_{j}")
                nc.scalar.activation(s_sb[:], g_ps[:], act_fn.Sigmoid)
                su_sb = silup.tile([P, cw], F32, tag="su", name=f"su_{slot}_{j}")
                nc.vector.tensor_mul(su_sb[:], s_sb[:], u_ps[:])
                a_sb = actp.tile([P, cw], BF16, tag=f"act_{slot}_{j}", name=f"act_{slot}_{j}")
                nc.vector.tensor_mul(a_sb[:], su_sb[:], g_ps[:])
                act_tiles[(slot, j)] = a_sb

            def fold_acc(s):
                # fold down-proj chunk 0 (A slots) / 0..1 (B) for the slot-j
                # pairs whose act completed at step s-1, using w2a strips
                aslot = 0 if (s - 1) < NJQ else 1
                aj = (s - 1) % NJQ
                for slot, j, strip0 in ((aslot, aj, 0), (2, s - 1, P)):
                    _, _, cw, nacc = SLOTS[slot]
                    for c in range(nacc):
                        nc.tensor.matmul(
                            acc[(slot, c)][:],
                            w2a_tiles[s - 1][:, strip0 + c * P:strip0 + (c + 1) * P],
                            act_tiles[(slot, j)][:],
                            start=(j == 0),
                            stop=(j == (NJQ - 1 if slot < 2 else NJH - 1)),
                        )

            # ---- GU phase
            for s in range(NSTEP):
                if s + 2 < NSTEP:
                    for ti in range(4 * (s + 2), 4 * (s + 2) + 4):
                        dma_w13(ti)
                    dma_w2a(s + 2)
                if s == 6:
                    dma_xt(xt2_tiles, xt2_d, CA2, "xt2", 0, 2)
                if s == 7:
                    dma_xt(xt2_tiles, xt2_d, CA2, "xt2", 1, 2)
                if 8 <= s < 12:
                    dma_w2b(0, s - 8)      # A1 w2b pairs
                if 12 <= s < 16:
                    dma_w2b(1, s - 12)     # A2 w2b pairs

                if s < NJQ:
                    gu_slot(0, CA1, s, 4 * s, 4 * s + 1)
                else:
                    gu_slot(1, CA2, s - NJQ, 4 * s, 4 * s + 1)
                gu_slot(2, CB, s, 4 * s + 2, 4 * s + 3)
                if s >= 1:
                    fold_acc(s)
            fold_acc(NSTEP)

            def writeback(ps, h, c0, cw, name, eng=None):
                o_sb = outp.tile([P, cw], BF16, tag="o", name=f"o_{name}")
                nc.vector.tensor_copy(o_sb[:], ps[:])
                # gpsimd queue keeps writebacks off the input-stream FIFO;
                # B-phase chunks ride sync (stream empty by then)
                (eng or nc.gpsimd).dma_start(y_d[h][:, c0:c0 + cw], o_sb[:])

            # w2b_B lands during the A down phases
            for jp in range(NJH // 2):
                dma_w2b(2, jp)
            for slot, c0, cw, nacc in SLOTS:
                for c in range(nacc):
                    writeback(acc[(slot, c)], c, c0, cw, f"acc{slot}{c}")

            # ---- DOWN phase: remaining chunks, slot-major, j ascending
            for slot, c0, cw, nacc in SLOTS:
                nj = NJQ if slot < 2 else NJH
                tags = ["g", "g", "u", "u"] + [f"acc{slot}{c}" for c in range(nacc)]
                for hi, h in enumerate(range(nacc, NH)):
                    tag = tags[hi % len(tags)]
                    ps = (psacc if tag.startswith("acc") else psgu).tile(
                        [P, cw], F32, tag=tag, name=f"yd_{slot}_{h}"
                    )
                    for j in range(nj):
                        nc.tensor.matmul(
                            ps[:],
                            w2b_tiles[(slot, j // 2)][:, j % 2, (h - nacc) * P:(h - nacc + 1) * P],
                            act_tiles[(slot, j)][:],
                            start=(j == 0),
                            stop=(j == nj - 1),
                        )
                    writeback(ps, h, c0, cw, f"d{slot}_{h}",
                              eng=nc.sync if slot == 2 else None)
    nc.compile()
    return nc


def _route(router_logits: np.ndarray, top_k: int):
    """Match jax.nn.softmax + jax.lax.top_k + renormalize (ties -> lower idx)."""
    p = router_logits.astype(np.float64)
    p = np.exp(p - p.max(axis=-1, keepdims=True))
    p /= p.sum(axis=-1, keepdims=True)
    order = np.argsort(-p, axis=-1, kind="stable")
    idx = order[:, :top_k]
    w = np.take_along_axis(p, idx, axis=-1)
    w /= w.sum(axis=-1, keepdims=True)
    return idx, w


def _pad4(n: int) -> int:
    return max(16, -(-n // 4) * 4)


def kernel(hidden_states, router_logits, W13, W2, top_k):
    global last_results
    top_k = int(top_k)
    hs = np.asarray(hidden_states, dtype=np.float32)
    T = hs.shape[0]
    idx, w = _route(np.asarray(router_logits, dtype=np.float32), top_k)

    tok_ids, tok_w = [], []
    for e in range(E):
        sel = idx == e  # [T, k]; at most one True per row
        rows = np.nonzero(sel.any(axis=-1))[0]
        tok_ids.append(rows)
        tok_w.append(w[sel].astype(np.float32))  # row-major -> token order

    loads = np.array([len(r) for r in tok_ids])
    order = np.argsort(-loads, kind="stable")
    big = [int(x) for x in order[:4]]    # quarter-I shards
    small = [int(x) for x in order[4:]]  # half-I shards
    CA1 = _pad4(max(loads[big[0]], loads[big[1]]))
    CA2 = _pad4(max(loads[big[2]], loads[big[3]]))
    CB = _pad4(max(loads[e] for e in small))
    assert CA1 <= 512 and CB <= 512, "token capacity exceeds one PSUM bank"
    CT = CA1 + CA2 + CB

    W13 = np.asarray(W13, dtype=np.float32)
    W2 = np.asarray(W2, dtype=np.float32)
    hsb = hs.astype(ml_dtypes.bfloat16)

    def w13_shard(e, q0, nj):
        # rows [gate q-chunk; up q-chunk] -> [2nj, P, KO, P], partition=h-col
        wg = W13[e][q0:q0 + nj * P]
        wu = W13[e][I + q0:I + q0 + nj * P]
        both = np.concatenate([wg, wu], axis=0).astype(ml_dtypes.bfloat16)
        return np.ascontiguousarray(both.reshape(2 * nj, P, KO, P).transpose(0, 3, 2, 1))

    def w2_shard(e, q0, nj):
        w2h = W2[e][:, q0:q0 + nj * P].astype(ml_dtypes.bfloat16)
        return np.ascontiguousarray(w2h.reshape(H, nj, P).transpose(1, 2, 0))

    def xt_arr(e, cap):
        xt = np.zeros((P, KO, cap), dtype=ml_dtypes.bfloat16)
        rows = tok_ids[e]
        n_e = len(rows)
        if n_e:
            xg = hsb[rows]
            xt[:, :, :n_e] = xg.T.reshape(KO, P, n_e).transpose(1, 0, 2)
        return xt

    in_maps = []
    core_slots = []  # per core: [(expert, col offset, n partial-shards), ...]
    for core in range(N_CORES):
        q = core % 4
        e1 = big[core // 4]          # A1 expert, quarter q
        e2 = big[2 + core // 4]      # A2 expert, quarter q
        ebs = small[core // 2]       # B expert, half core%2
        sh1 = (e1, q * IQ, NJQ)
        sh2 = (e2, q * IQ, NJQ)
        shb = (ebs, (core % 2) * IH, NJH)
        core_slots.append([(e1, 0, CA1), (e2, CA1, CA2), (ebs, CA1 + CA2, CB)])

        w2_1 = w2_shard(*sh1)   # [8, P, H]
        w2_2 = w2_shard(*sh2)
        w2_b = w2_shard(*shb)   # [16, P, H]
        # w13 flat tiles in consumption order: step s -> [A-g, A-u, B-g, B-u]
        w13_1 = w13_shard(*sh1)  # [16, P, KO, P]: gate 0..7, up 8..15
        w13_2 = w13_shard(*sh2)
        w13_b = w13_shard(*shb)  # [32, ...]: gate 0..15, up 16..31
        flat = np.empty((NSTEP * 4, P, KO, P), dtype=ml_dtypes.bfloat16)
        for s in range(NSTEP):
            wa = w13_1 if s < NJQ else w13_2
            ja = s % NJQ
            flat[4 * s + 0] = wa[ja]
            flat[4 * s + 1] = wa[NJQ + ja]
            flat[4 * s + 2] = w13_b[s]
            flat[4 * s + 3] = w13_b[NJH + s]
        # w2a strips per step: A-slot 128 cols + B 256 cols
        w2a = np.empty((NSTEP, P, 3 * P), dtype=ml_dtypes.bfloat16)
        for s in range(NSTEP):
            wa = w2_1 if s < NJQ else w2_2
            w2a[s, :, :P] = wa[s % NJQ][:, :P]
            w2a[s, :, P:] = w2_b[s][:, :2 * P]

        def pair_pack(w2t, nc1):
            # [nj, P, H-nc1*P] -> [nj//2, P, 2, W]
            t = w2t[:, :, nc1 * P:]
            nj = t.shape[0]
            return np.ascontiguousarray(
                t.reshape(nj // 2, 2, P, t.shape[2]).transpose(0, 2, 1, 3))

        in_maps.append({
            "xt1": xt_arr(e1, CA1),
            "xt2": xt_arr(e2, CA2),
            "xtb": xt_arr(ebs, CB),
            "w13": flat,
            "w2a": w2a,
            "w2b1": pair_pack(w2_1, 1),
            "w2b2": pair_pack(w2_2, 1),
            "w2bb": pair_pack(w2_b, 2),
        })

    nc = _build_nc(CA1, CA2, CB)
    res = run_bass_kernel_spmd(
        nc,
        in_maps,
        list(range(N_CORES)),
        trace=bool(os.environ.get("MOE_TRACE")),
        tmpdir=os.environ.get("MOE_TRACE_DIR") or None,
    )
    last_results = res

    out = np.zeros((T, H), dtype=np.float32)
    for core in range(N_CORES):
        y = res.results[core]["y"].reshape(H, CT).astype(np.float32)
        for (e, c0, cap) in core_slots[core]:
            rows = tok_ids[e]
            n_e = len(rows)
            if n_e:
                out[rows] += y[:, c0:c0 + n_e].T * tok_w[e][:, None]
    return out
